# revision 1
# baseline (speedup 1.0000x reference)
"""DIN attention kernel for Trainium2, 8 NeuronCores, data-parallel over batch.

Reference computation (per batch element b):
    x[s]  = concat(t, h[s], t-h[s], t*h[s])          # [S, 4D]
    a     = x @ W1 + b1 ; h1 = relu(a)               # [S, H]
    w     = h1 @ W2 + b2                             # [S]
    w     = w*m + (-1e9)(1-m) ; p = softmax(w)       # [S]
    out   = p @ h                                    # [D]

Algebraic restructuring used here:
    x@W1 = t@Wt + h@Wh + (t*h)@Wp
      with Wt = W1[0:D]+W1[2D:3D], Wh = W1[D:2D]-W1[2D:3D], Wp = W1[3D:4D]
    (t*h)@Wp = h @ (diag(t) Wp)   -> per-b scaled weight, no explicit t*h
    u_b = Wt.T t_b + b1 enters as the relu bias.
    b2 is dropped (softmax shift-invariant); 1/Z folded into final scale.
    Masking: w + pen with pen = (m-1)*1e9  (exp underflows to 0 where m=0).

Layout strategy: history is cast-loaded (f32->bf16) in its natural [s, d]
layout (weighted-sum stationary), and transposed on the PE (identity matmul)
into [d, s] for the MLP matmuls. Transposes are packed 8-per-PSUM-bank so a
single DVE/ACT copy evacuates 8 batch elements at once. Scores land in PSUM
as [s, b] columns, are PE-transposed to [b, s] for the masked softmax, and
the exp weights are PE-transposed back to [s, b] for the weighted sum.
"""

import numpy as np
import ml_dtypes

import bass_rust
import concourse.tile as tile
import concourse.mybir as mybir
from concourse import bacc
from concourse.bass_utils import run_bass_kernel_spmd

F32 = mybir.dt.float32
BF16 = mybir.dt.bfloat16
AX = mybir.AxisListType
ALU = mybir.AluOpType
ACTF = mybir.ActivationFunctionType

B, S, D, H = 2048, 200, 128, 64
N_CORES = 8
SC0 = 128          # first s-chunk
SC1 = S - SC0      # 72


def build_nc(Bc=256, BT=128, NG=8):
    """Build the per-core Bass program. Bc = batch rows per core,
    BT = softmax tile (<=128), NG = history load-group size."""
    assert Bc % BT == 0 and BT % NG == 0
    n_tiles = Bc // BT
    n_groups = BT // NG
    QB = min(4, NG)    # transposes packed per PSUM bank
    assert NG % QB == 0

    nc = bacc.Bacc("TRN2", debug=False, target_bir_lowering=False)

    hist = nc.dram_tensor("hist", [Bc, S, D], F32, kind="ExternalInput").ap()
    tgt = nc.dram_tensor("tgt", [Bc, D], F32, kind="ExternalInput").ap()
    pen = nc.dram_tensor("pen", [Bc, S], F32, kind="ExternalInput").ap()
    wt_d = nc.dram_tensor("wt", [D, H], BF16, kind="ExternalInput").ap()
    wh_d = nc.dram_tensor("wh", [D, H], BF16, kind="ExternalInput").ap()
    wp_d = nc.dram_tensor("wp", [D, H], BF16, kind="ExternalInput").ap()
    b1_d = nc.dram_tensor("b1", [2 * H, 1], F32, kind="ExternalInput").ap()
    w2_d = nc.dram_tensor("w2", [2 * H, 1], BF16, kind="ExternalInput").ap()
    idf_d = nc.dram_tensor("idf", [128, 128], F32, kind="ExternalInput").ap()
    idb_d = nc.dram_tensor("idb", [128, 128], BF16, kind="ExternalInput").ap()
    out = nc.dram_tensor("out", [Bc, D], F32, kind="ExternalOutput").ap()

    from contextlib import ExitStack
    with tile.TileContext(nc) as tc, ExitStack() as stack:
        consts = stack.enter_context(tc.tile_pool(name="consts", bufs=1))
        wt_s = consts.tile([D, H], BF16)
        nc.sync.dma_start(out=wt_s, in_=wt_d)
        wh_s = consts.tile([D, H], BF16)
        nc.sync.dma_start(out=wh_s, in_=wh_d)
        wp_s = consts.tile([D, H], BF16)
        nc.sync.dma_start(out=wp_s, in_=wp_d)
        b1_s = consts.tile([2 * H, 1], F32)
        nc.sync.dma_start(out=b1_s, in_=b1_d)
        w2_s = consts.tile([2 * H, 1], BF16)
        nc.sync.dma_start(out=w2_s, in_=w2_d)
        idf_s = consts.tile([128, 128], F32)
        nc.sync.dma_start(out=idf_s, in_=idf_d)
        idb_s = consts.tile([128, 128], BF16)
        nc.sync.dma_start(out=idb_s, in_=idb_d)

        tilep = stack.enter_context(tc.tile_pool(name="tilep", bufs=2))
        hn0p = stack.enter_context(tc.tile_pool(name="hn0p", bufs=n_groups + 8))
        hn1p = stack.enter_context(tc.tile_pool(name="hn1p", bufs=n_groups + 8))
        htp = stack.enter_context(tc.tile_pool(name="htp", bufs=3))
        h1p = stack.enter_context(tc.tile_pool(name="h1p", bufs=8))
        wpbp = stack.enter_context(tc.tile_pool(name="wpbp", bufs=8))
        smallp = stack.enter_context(tc.tile_pool(name="smallp", bufs=6))

        # PSUM budget (8 banks): psa 3 + acc 2 + trx 3 (trx slots shared with transients)
        psa = stack.enter_context(tc.tile_pool(name="psa", bufs=3, space="PSUM"))
        accp = stack.enter_context(tc.tile_pool(name="accp", bufs=2, space="PSUM"))
        trxp = stack.enter_context(tc.tile_pool(name="trxp", bufs=3, space="PSUM"))

        def emit_tile_prep(tt):
            st = {}
            b0 = tt * BT
            # per-tile target prep: tT [D, BT] (f32 + bf16), U2 stacked pairs
            tgt_sb = smallp.tile([BT, D], F32, tag="tgt_sb")
            nc.sync.dma_start(out=tgt_sb, in_=tgt[b0:b0 + BT, :])
            ps_t = trxp.tile([D, BT], F32, tag="trx")
            nc.tensor.transpose(ps_t, tgt_sb, idf_s[0:BT, 0:BT])
            tT = tilep.tile([D, BT], F32, tag="tT")
            nc.vector.tensor_copy(tT, ps_t)
            tTb = tilep.tile([D, BT], BF16, tag="tTb")
            nc.vector.tensor_copy(tTb, ps_t)
            ps_u2 = trxp.tile([2 * H, BT // 2], F32, tag="trx")
            mu0 = nc.tensor.matmul(ps_u2[0:H, :], lhsT=wt_s,
                                   rhs=tTb[:, 0:BT:2],
                                   start=True, stop=False,
                                   tile_position=(0, 0),
                                   skip_group_check=True)
            mu1 = nc.tensor.matmul(ps_u2[H:2 * H, :], lhsT=wt_s,
                                   rhs=tTb[:, 1:BT:2],
                                   start=True, stop=True,
                                   tile_position=(0, H),
                                   skip_group_check=True)
            bass_rust.add_dep_helper(mu1.ins, mu0.ins,
                                     reason="psum half-bank group order")
            U2 = tilep.tile([2 * H, BT // 2], F32, tag="U2")
            nc.vector.tensor_scalar_add(U2, ps_u2, b1_s)
            pen_t = tilep.tile([BT, S], F32, tag="pen_t")
            nc.sync.dma_start(out=pen_t, in_=pen[b0:b0 + BT, :])
            # accumulator bank: scores [s,b] x2 + output [d,b]
            acc = accp.tile([128, 3, BT], F32, tag="acc")
            nc.vector.memset(acc, 0.0)
            st.update(tT=tT, U2=U2, pen_t=pen_t, acc=acc,
                      hn0=[None] * n_groups, hn1=[None] * n_groups)
            return st

        def emit_group_pass1(st, tt, g):
            b0 = tt * BT
            tT, U2, acc = st["tT"], st["U2"], st["acc"]
            gb = b0 + g * NG
            hn0 = hn0p.tile([SC0, NG, D], BF16, tag="hn0")
            nc.gpsimd.dma_start(
                out=hn0,
                in_=hist[gb:gb + NG, 0:SC0, :].rearrange("b s d -> s b d"),
            )
            hn1 = hn1p.tile([SC1, NG, D], BF16, tag="hn1")
            nc.gpsimd.dma_start(
                out=hn1,
                in_=hist[gb:gb + NG, SC0:S, :].rearrange("b s d -> s b d"),
            )
            st["hn0"][g] = hn0
            st["hn1"][g] = hn1

            # PE transposes: QB b's (both s-chunks) packed per PSUM bank,
            # one batched evacuation copy per pack
            hT = htp.tile([D, NG, S], BF16, tag="hT")
            for q in range(0, NG, QB):
                trx = trxp.tile([D, QB, S], BF16, tag="trx")
                prev = None
                for jj in range(QB):
                    m_a = nc.tensor.matmul(
                        trx[:, jj, 0:SC0], lhsT=hn0[:, q + jj, :],
                        rhs=idb_s,
                        start=(jj == 0), stop=False,
                        is_transpose=True, skip_group_check=True)
                    if prev is not None:
                        bass_rust.add_dep_helper(
                            m_a.ins, prev.ins, reason="trx pack order")
                    m_b = nc.tensor.matmul(
                        trx[:, jj, SC0:S], lhsT=hn1[:, q + jj, :],
                        rhs=idb_s[0:SC1, 0:SC1],
                        start=False, stop=(jj == QB - 1),
                        is_transpose=True, skip_group_check=True)
                    bass_rust.add_dep_helper(
                        m_b.ins, m_a.ins, reason="trx pack order")
                    prev = m_b
                if (q // QB) % 2 == 0:
                    nc.vector.tensor_copy(hT[:, q:q + QB, :], trx)
                else:
                    nc.scalar.copy(hT[:, q:q + QB, :], trx)

            for j in range(0, NG, 2):
                jb = g * NG + j          # even b of the pair
                p = jb // 2              # pair index within tile
                wpb0 = wpbp.tile([D, H], BF16, tag="wpb")
                nc.vector.tensor_scalar_mul(wpb0, wp_s, tT[:, jb:jb + 1])
                wpb1 = wpbp.tile([D, H], BF16, tag="wpb")
                nc.vector.tensor_scalar_mul(wpb1, wp_s, tT[:, jb + 1:jb + 2])
                rhs0 = hT[:, j, 0:S]
                rhs1 = hT[:, j + 1, 0:S]
                # two b's share one PSUM bank: rows 0-63 / 64-127
                ps_a2 = psa.tile([2 * H, S], F32, tag="ps_a")
                ma0 = nc.tensor.matmul(ps_a2[0:H, :], lhsT=wh_s, rhs=rhs0,
                                       start=True, stop=False,
                                       tile_position=(0, 0),
                                       skip_group_check=True)
                ma1 = nc.tensor.matmul(ps_a2[0:H, :], lhsT=wpb0, rhs=rhs0,
                                       start=False, stop=False,
                                       tile_position=(0, 0),
                                       skip_group_check=True)
                ma2 = nc.tensor.matmul(ps_a2[H:2 * H, :], lhsT=wh_s,
                                       rhs=rhs1,
                                       start=True, stop=False,
                                       tile_position=(0, H),
                                       skip_group_check=True)
                nc.tensor.matmul(ps_a2[H:2 * H, :], lhsT=wpb1, rhs=rhs1,
                                 start=False, stop=True,
                                 tile_position=(0, H),
                                 skip_group_check=True)
                bass_rust.add_dep_helper(ma2.ins, ma1.ins,
                                         reason="psum half-bank group order")
                bass_rust.add_dep_helper(ma1.ins, ma0.ins,
                                         reason="psum accum order")
                # one relu handles both b's (bias col = stacked u's)
                h1 = h1p.tile([2 * H, S], BF16, tag="h1")
                if p % 2 == 0:
                    nc.scalar.activation(h1, ps_a2, ACTF.Relu,
                                         bias=U2[:, p:p + 1])
                else:
                    nc.vector.tensor_scalar(
                        h1, ps_a2, scalar1=U2[:, p:p + 1], scalar2=0.0,
                        op0=ALU.add, op1=ALU.max)
                # scores: row-tiled matmuls, K=64 each half
                nc.tensor.matmul(acc[:, 0, jb:jb + 1],
                                 lhsT=h1[0:H, 0:SC0], rhs=w2_s[0:H],
                                 start=False, stop=True,
                                 tile_position=(0, 0),
                                 skip_group_check=True)
                nc.tensor.matmul(acc[0:SC1, 1, jb:jb + 1],
                                 lhsT=h1[0:H, SC0:S], rhs=w2_s[0:H],
                                 start=False, stop=True,
                                 tile_position=(0, 0),
                                 skip_group_check=True)
                nc.tensor.matmul(acc[:, 0, jb + 1:jb + 2],
                                 lhsT=h1[H:2 * H, 0:SC0],
                                 rhs=w2_s[H:2 * H],
                                 start=False, stop=True,
                                 tile_position=(H, 0),
                                 skip_group_check=True)
                nc.tensor.matmul(acc[0:SC1, 1, jb + 1:jb + 2],
                                 lhsT=h1[H:2 * H, SC0:S],
                                 rhs=w2_s[H:2 * H],
                                 start=False, stop=True,
                                 tile_position=(H, 0),
                                 skip_group_check=True)

        def emit_softmax(st):
            acc, pen_t = st["acc"], st["pen_t"]
            w0s = tilep.tile([SC0, BT], F32, tag="w0s")
            nc.vector.tensor_copy(w0s, acc[:, 0, :])
            w1s = tilep.tile([SC1, BT], F32, tag="w1s")
            nc.scalar.copy(w1s, acc[0:SC1, 1, :])
            ps_x0 = trxp.tile([BT, SC0], F32, tag="trx")
            nc.tensor.transpose(ps_x0, w0s, idf_s[0:SC0, 0:SC0])
            ps_x1 = trxp.tile([BT, SC1], F32, tag="trx")
            nc.tensor.transpose(ps_x1, w1s, idf_s[0:SC1, 0:SC1])
            wbs = tilep.tile([BT, S], F32, tag="wbs")
            nc.vector.tensor_copy(wbs[:, 0:SC0], ps_x0)
            nc.scalar.copy(wbs[:, SC0:S], ps_x1)
            nc.vector.tensor_add(wbs, wbs, pen_t)
            nmx = smallp.tile([BT, 1], F32, tag="nmx")
            nc.vector.tensor_reduce(nmx, wbs, axis=AX.X, op=ALU.max,
                                    negate=True)
            ebs = tilep.tile([BT, S], BF16, tag="ebs")
            zs = smallp.tile([BT, 1], F32, tag="zs")
            nc.scalar.activation(ebs, wbs, ACTF.Exp, bias=nmx, accum_out=zs)
            rz = smallp.tile([BT, 1], F32, tag="rz")
            nc.vector.reciprocal(rz, zs)
            # e transposed back to [s, b] columns for the weighted sum
            ps_e0 = trxp.tile([SC0, BT], BF16, tag="trx")
            nc.tensor.transpose(ps_e0, ebs[:, 0:SC0], idb_s[0:BT, 0:BT])
            ps_e1 = trxp.tile([SC1, BT], BF16, tag="trx")
            nc.tensor.transpose(ps_e1, ebs[:, SC0:S], idb_s[0:BT, 0:BT])
            eT0 = tilep.tile([SC0, BT], BF16, tag="eT0")
            nc.vector.tensor_copy(eT0, ps_e0)
            eT1 = tilep.tile([SC1, BT], BF16, tag="eT1")
            nc.scalar.copy(eT1, ps_e1)
            st.update(eT0=eT0, eT1=eT1, rz=rz)

        def emit_wsum_group(st, g):
            acc, eT0, eT1 = st["acc"], st["eT0"], st["eT1"]
            hn0, hn1 = st["hn0"][g], st["hn1"][g]
            for j in range(NG):
                jb = g * NG + j
                nc.tensor.matmul(acc[:, 2, jb:jb + 1], lhsT=hn0[:, j, :],
                                 rhs=eT0[:, jb:jb + 1], start=False,
                                 stop=False, skip_group_check=True)
                nc.tensor.matmul(acc[:, 2, jb:jb + 1], lhsT=hn1[:, j, :],
                                 rhs=eT1[:, jb:jb + 1], start=False,
                                 stop=True, skip_group_check=True)

        def emit_output(st, tt):
            b0 = tt * BT
            acc, rz = st["acc"], st["rz"]
            oT = tilep.tile([D, BT], F32, tag="oT")
            nc.vector.tensor_copy(oT, acc[:, 2, :])
            ps_ot = trxp.tile([BT, D], F32, tag="trx")
            nc.tensor.transpose(ps_ot, oT, idf_s[0:D, 0:D])
            ofin = tilep.tile([BT, D], F32, tag="ofin")
            nc.vector.tensor_scalar_mul(ofin, ps_ot, rz)
            nc.sync.dma_start(out=out[b0:b0 + BT, :], in_=ofin)

        # ---- software pipeline over tiles: overlap tile t's weighted-sum
        # with tile t+1's load/transpose/MLP/score groups
        st_cur = emit_tile_prep(0)
        for g in range(n_groups):
            emit_group_pass1(st_cur, 0, g)
        emit_softmax(st_cur)
        for tt in range(n_tiles):
            if tt + 1 < n_tiles:
                st_next = emit_tile_prep(tt + 1)
                for g in range(n_groups):
                    emit_wsum_group(st_cur, g)
                    emit_group_pass1(st_next, tt + 1, g)
                emit_output(st_cur, tt)
                emit_softmax(st_next)
                st_cur = st_next
            else:
                for g in range(n_groups):
                    emit_wsum_group(st_cur, g)
                emit_output(st_cur, tt)

    nc.compile()
    return nc


_CACHE = {}


def _get_nc(Bc=256, BT=128, NG=8):
    key = (Bc, BT, NG)
    if key not in _CACHE:
        _CACHE[key] = build_nc(Bc, BT, NG)
    return _CACHE[key]


def make_in_maps(target_item, history_sequence, mask, W1, b1, W2, b2,
                 n_cores=N_CORES):
    """Host-side prep: factored weights, penalty array, per-core shards."""
    f32 = np.float32
    bf16 = ml_dtypes.bfloat16
    W1 = np.asarray(W1, f32)
    wt = (W1[0:D] + W1[2 * D:3 * D]).astype(bf16)
    wh = (W1[D:2 * D] - W1[2 * D:3 * D]).astype(bf16)
    wp = W1[3 * D:4 * D].astype(bf16)
    b1v = np.asarray(b1, f32).reshape(H)
    b1c = np.concatenate([b1v, b1v]).reshape(2 * H, 1)
    w2v = np.asarray(W2, f32).reshape(H)
    w2c = np.concatenate([w2v, w2v]).astype(bf16).reshape(2 * H, 1)
    idf = np.eye(128, dtype=f32)
    idb = np.eye(128).astype(bf16)
    pen_full = ((np.asarray(mask, f32) - 1.0) * 1e9).astype(f32)
    tgt_full = np.asarray(target_item, f32)
    hist_full = np.asarray(history_sequence, f32)

    shared = dict(wt=wt, wh=wh, wp=wp, b1=b1c, w2=w2c, idf=idf, idb=idb)
    Bc = tgt_full.shape[0] // n_cores
    in_maps = []
    for c in range(n_cores):
        sl = slice(c * Bc, (c + 1) * Bc)
        in_maps.append(dict(hist=hist_full[sl], tgt=tgt_full[sl],
                            pen=pen_full[sl], **shared))
    return in_maps


def kernel(target_item, history_sequence, mask, W1, b1, W2, b2):
    nc = _get_nc()
    in_maps = make_in_maps(target_item, history_sequence, mask, W1, b1, W2, b2)
    res = run_bass_kernel_spmd(nc, in_maps, list(range(N_CORES)))
    return np.concatenate([res.results[c]["out"] for c in range(N_CORES)],
                          axis=0)



# revision 6
# speedup vs baseline: 5.8246x; 5.8246x over previous
"""DIN attention kernel for Trainium2, 8 NeuronCores, data-parallel over batch.

Reference computation (per batch element b):
    x[s]  = concat(t, h[s], t-h[s], t*h[s])          # [S, 4D]
    a     = x @ W1 + b1 ; h1 = relu(a)               # [S, H]
    w     = h1 @ W2 + b2                             # [S]
    w     = w*m + (-1e9)(1-m) ; p = softmax(w)       # [S]
    out   = p @ h                                    # [D]

Algebraic restructuring used here:
    x@W1 = t@Wt + h@Wh + (t*h)@Wp
      with Wt = W1[0:D]+W1[2D:3D], Wh = W1[D:2D]-W1[2D:3D], Wp = W1[3D:4D]
    (t*h)@Wp = h @ (diag(t) Wp)   -> per-b scaled weight, no explicit t*h
    u_b = Wt.T t_b + b1 enters as the relu bias.
    b2 is dropped (softmax shift-invariant); 1/Z folded into final scale.
    Masking: w + pen with pen = (m-1)*1e9  (exp underflows to 0 where m=0).

Layout strategy: history is cast-loaded (f32->bf16) in its natural [s, d]
layout (weighted-sum stationary), and transposed on the PE (identity matmul)
into [d, s] for the MLP matmuls. Transposes are packed 8-per-PSUM-bank so a
single DVE/ACT copy evacuates 8 batch elements at once. Scores land in PSUM
as [s, b] columns, are PE-transposed to [b, s] for the masked softmax, and
the exp weights are PE-transposed back to [s, b] for the weighted sum.
"""

import numpy as np
import ml_dtypes

import bass_rust
import concourse.tile as tile
import concourse.mybir as mybir
from concourse import bacc
from concourse.bass_utils import run_bass_kernel_spmd

F32 = mybir.dt.float32
BF16 = mybir.dt.bfloat16
AX = mybir.AxisListType
ALU = mybir.AluOpType
ACTF = mybir.ActivationFunctionType

B, S, D, H = 2048, 200, 128, 64
N_CORES = 8
SC0 = 128          # first s-chunk
SC1 = S - SC0      # 72


def build_nc(Bc=256, BT=128, NG=8):
    """Build the per-core Bass program. Bc = batch rows per core,
    BT = softmax tile (<=128), NG = history load-group size."""
    assert Bc % BT == 0 and BT % NG == 0
    n_tiles = Bc // BT
    n_groups = BT // NG
    QB = min(4, NG)    # transposes packed per PSUM bank
    assert NG % QB == 0

    nc = bacc.Bacc("TRN2", debug=False, target_bir_lowering=False)

    hist = nc.dram_tensor("hist", [Bc, S, D], BF16, kind="ExternalInput").ap()
    tgt = nc.dram_tensor("tgt", [Bc, D], F32, kind="ExternalInput").ap()
    pen = nc.dram_tensor("pen", [Bc, S], F32, kind="ExternalInput").ap()
    wt_d = nc.dram_tensor("wt", [D, H], BF16, kind="ExternalInput").ap()
    wh_d = nc.dram_tensor("wh", [D, H], BF16, kind="ExternalInput").ap()
    wp_d = nc.dram_tensor("wp", [D, H], BF16, kind="ExternalInput").ap()
    b1_d = nc.dram_tensor("b1", [2 * H, 1], F32, kind="ExternalInput").ap()
    w2_d = nc.dram_tensor("w2", [2 * H, 1], BF16, kind="ExternalInput").ap()
    idf_d = nc.dram_tensor("idf", [128, 128], F32, kind="ExternalInput").ap()
    idb_d = nc.dram_tensor("idb", [128, 128], BF16, kind="ExternalInput").ap()
    out = nc.dram_tensor("out", [Bc, D], F32, kind="ExternalOutput").ap()

    from contextlib import ExitStack
    with tile.TileContext(nc) as tc, ExitStack() as stack:
        consts = stack.enter_context(tc.tile_pool(name="consts", bufs=1))
        wt_s = consts.tile([D, H], BF16)
        nc.sync.dma_start(out=wt_s, in_=wt_d)
        wh_s = consts.tile([D, H], BF16)
        nc.sync.dma_start(out=wh_s, in_=wh_d)
        wp_s = consts.tile([D, H], BF16)
        nc.sync.dma_start(out=wp_s, in_=wp_d)
        b1_s = consts.tile([2 * H, 1], F32)
        nc.sync.dma_start(out=b1_s, in_=b1_d)
        w2_s = consts.tile([2 * H, 1], BF16)
        nc.sync.dma_start(out=w2_s, in_=w2_d)
        idf_s = consts.tile([128, 128], F32)
        nc.sync.dma_start(out=idf_s, in_=idf_d)
        idb_s = consts.tile([128, 128], BF16)
        nc.sync.dma_start(out=idb_s, in_=idb_d)

        tilep = stack.enter_context(tc.tile_pool(name="tilep", bufs=2))
        hn0p = stack.enter_context(tc.tile_pool(name="hn0p", bufs=n_groups + 8))
        hn1p = stack.enter_context(tc.tile_pool(name="hn1p", bufs=n_groups + 8))
        htp = stack.enter_context(tc.tile_pool(name="htp", bufs=3))
        h1p = stack.enter_context(tc.tile_pool(name="h1p", bufs=8))
        wpbp = stack.enter_context(tc.tile_pool(name="wpbp", bufs=8))
        smallp = stack.enter_context(tc.tile_pool(name="smallp", bufs=6))

        # PSUM budget (8 banks): psa 3 + acc 2 + trx 3 (trx slots shared with transients)
        psa = stack.enter_context(tc.tile_pool(name="psa", bufs=3, space="PSUM"))
        accp = stack.enter_context(tc.tile_pool(name="accp", bufs=2, space="PSUM"))
        trxp = stack.enter_context(tc.tile_pool(name="trxp", bufs=3, space="PSUM"))

        def emit_tile_prep(tt):
            st = {}
            b0 = tt * BT
            # per-tile target prep: tT [D, BT] (f32 + bf16), U2 stacked pairs
            tgt_sb = smallp.tile([BT, D], F32, tag="tgt_sb")
            nc.sync.dma_start(out=tgt_sb, in_=tgt[b0:b0 + BT, :])
            ps_t = trxp.tile([D, BT], F32, tag="trx")
            nc.tensor.transpose(ps_t, tgt_sb, idf_s[0:BT, 0:BT])
            tT = tilep.tile([D, BT], F32, tag="tT")
            nc.vector.tensor_copy(tT, ps_t)
            tTb = tilep.tile([D, BT], BF16, tag="tTb")
            nc.vector.tensor_copy(tTb, ps_t)
            ps_u2 = trxp.tile([2 * H, BT // 2], F32, tag="trx")
            mu0 = nc.tensor.matmul(ps_u2[0:H, :], lhsT=wt_s,
                                   rhs=tTb[:, 0:BT:2],
                                   start=True, stop=False,
                                   tile_position=(0, 0),
                                   skip_group_check=True)
            mu1 = nc.tensor.matmul(ps_u2[H:2 * H, :], lhsT=wt_s,
                                   rhs=tTb[:, 1:BT:2],
                                   start=True, stop=True,
                                   tile_position=(0, H),
                                   skip_group_check=True)
            bass_rust.add_dep_helper(mu1.ins, mu0.ins,
                                     reason="psum half-bank group order")
            U2 = tilep.tile([2 * H, BT // 2], F32, tag="U2")
            nc.vector.tensor_scalar_add(U2, ps_u2, b1_s)
            pen_t = tilep.tile([BT, S], F32, tag="pen_t")
            nc.sync.dma_start(out=pen_t, in_=pen[b0:b0 + BT, :])
            # accumulator bank: scores [s,b] x2 + output [d,b]
            acc = accp.tile([128, 3, BT], F32, tag="acc")
            nc.vector.memset(acc, 0.0)
            st.update(tT=tT, U2=U2, pen_t=pen_t, acc=acc,
                      hn0=[None] * n_groups, hn1=[None] * n_groups)
            return st

        def emit_group_pass1(st, tt, g):
            b0 = tt * BT
            tT, U2, acc = st["tT"], st["U2"], st["acc"]
            gb = b0 + g * NG
            hn0 = hn0p.tile([SC0, NG, D], BF16, tag="hn0")
            nc.gpsimd.dma_start(
                out=hn0,
                in_=hist[gb:gb + NG, 0:SC0, :].rearrange("b s d -> s b d"),
            )
            hn1 = hn1p.tile([SC1, NG, D], BF16, tag="hn1")
            nc.gpsimd.dma_start(
                out=hn1,
                in_=hist[gb:gb + NG, SC0:S, :].rearrange("b s d -> s b d"),
            )
            st["hn0"][g] = hn0
            st["hn1"][g] = hn1

            # PE transposes: QB b's (both s-chunks) packed per PSUM bank,
            # one batched evacuation copy per pack
            hT = htp.tile([D, NG, S], BF16, tag="hT")
            for q in range(0, NG, QB):
                trx = trxp.tile([D, QB, S], BF16, tag="trx")
                prev = None
                for jj in range(QB):
                    m_a = nc.tensor.matmul(
                        trx[:, jj, 0:SC0], lhsT=hn0[:, q + jj, :],
                        rhs=idb_s,
                        start=(jj == 0), stop=False,
                        is_transpose=True, skip_group_check=True)
                    if prev is not None:
                        bass_rust.add_dep_helper(
                            m_a.ins, prev.ins, reason="trx pack order")
                    m_b = nc.tensor.matmul(
                        trx[:, jj, SC0:S], lhsT=hn1[:, q + jj, :],
                        rhs=idb_s[0:SC1, 0:SC1],
                        start=False, stop=(jj == QB - 1),
                        is_transpose=True, skip_group_check=True)
                    bass_rust.add_dep_helper(
                        m_b.ins, m_a.ins, reason="trx pack order")
                    prev = m_b
                if (q // QB) % 2 == 0:
                    nc.vector.tensor_copy(hT[:, q:q + QB, :], trx)
                else:
                    nc.scalar.copy(hT[:, q:q + QB, :], trx)

            for j in range(0, NG, 2):
                jb = g * NG + j          # even b of the pair
                p = jb // 2              # pair index within tile
                # fused per-b weight: W_j = wh + t_j * wp  (one matmul per b)
                wpb0 = wpbp.tile([D, H], BF16, tag="wpb")
                nc.vector.scalar_tensor_tensor(
                    wpb0, wp_s, tT[:, jb:jb + 1], wh_s,
                    op0=ALU.mult, op1=ALU.add)
                wpb1 = wpbp.tile([D, H], BF16, tag="wpb")
                nc.vector.scalar_tensor_tensor(
                    wpb1, wp_s, tT[:, jb + 1:jb + 2], wh_s,
                    op0=ALU.mult, op1=ALU.add)
                rhs0 = hT[:, j, 0:S]
                rhs1 = hT[:, j + 1, 0:S]
                # two b's share one PSUM bank: rows 0-63 / 64-127
                ps_a2 = psa.tile([2 * H, S], F32, tag="ps_a")
                ma0 = nc.tensor.matmul(ps_a2[0:H, :], lhsT=wpb0, rhs=rhs0,
                                       start=True, stop=False,
                                       tile_position=(0, 0),
                                       skip_group_check=True)
                ma1 = nc.tensor.matmul(ps_a2[H:2 * H, :], lhsT=wpb1,
                                       rhs=rhs1,
                                       start=True, stop=True,
                                       tile_position=(0, H),
                                       skip_group_check=True)
                bass_rust.add_dep_helper(ma1.ins, ma0.ins,
                                         reason="psum half-bank group order")
                # one relu handles both b's (bias col = stacked u's)
                h1 = h1p.tile([2 * H, S], BF16, tag="h1")
                if p % 2 == 0:
                    nc.scalar.activation(h1, ps_a2, ACTF.Relu,
                                         bias=U2[:, p:p + 1])
                else:
                    nc.vector.tensor_scalar(
                        h1, ps_a2, scalar1=U2[:, p:p + 1], scalar2=0.0,
                        op0=ALU.add, op1=ALU.max)
                # scores: row-tiled matmuls, K=64 each half
                nc.tensor.matmul(acc[:, 0, jb:jb + 1],
                                 lhsT=h1[0:H, 0:SC0], rhs=w2_s[0:H],
                                 start=False, stop=True,
                                 tile_position=(0, 0),
                                 skip_group_check=True)
                nc.tensor.matmul(acc[0:SC1, 1, jb:jb + 1],
                                 lhsT=h1[0:H, SC0:S], rhs=w2_s[0:H],
                                 start=False, stop=True,
                                 tile_position=(0, 0),
                                 skip_group_check=True)
                nc.tensor.matmul(acc[:, 0, jb + 1:jb + 2],
                                 lhsT=h1[H:2 * H, 0:SC0],
                                 rhs=w2_s[H:2 * H],
                                 start=False, stop=True,
                                 tile_position=(H, 0),
                                 skip_group_check=True)
                nc.tensor.matmul(acc[0:SC1, 1, jb + 1:jb + 2],
                                 lhsT=h1[H:2 * H, SC0:S],
                                 rhs=w2_s[H:2 * H],
                                 start=False, stop=True,
                                 tile_position=(H, 0),
                                 skip_group_check=True)

        def emit_softmax(st):
            acc, pen_t = st["acc"], st["pen_t"]
            w0s = tilep.tile([SC0, BT], F32, tag="w0s")
            nc.vector.tensor_copy(w0s, acc[:, 0, :])
            w1s = tilep.tile([SC1, BT], F32, tag="w1s")
            nc.scalar.copy(w1s, acc[0:SC1, 1, :])
            ps_x0 = trxp.tile([BT, SC0], F32, tag="trx")
            nc.tensor.transpose(ps_x0, w0s, idf_s[0:SC0, 0:SC0])
            ps_x1 = trxp.tile([BT, SC1], F32, tag="trx")
            nc.tensor.transpose(ps_x1, w1s, idf_s[0:SC1, 0:SC1])
            wbs = tilep.tile([BT, S], F32, tag="wbs")
            nc.vector.tensor_copy(wbs[:, 0:SC0], ps_x0)
            nc.scalar.copy(wbs[:, SC0:S], ps_x1)
            nc.vector.tensor_add(wbs, wbs, pen_t)
            nmx = smallp.tile([BT, 1], F32, tag="nmx")
            nc.vector.tensor_reduce(nmx, wbs, axis=AX.X, op=ALU.max,
                                    negate=True)
            ebs = tilep.tile([BT, S], BF16, tag="ebs")
            zs = smallp.tile([BT, 1], F32, tag="zs")
            nc.scalar.activation(ebs, wbs, ACTF.Exp, bias=nmx, accum_out=zs)
            rz = smallp.tile([BT, 1], F32, tag="rz")
            nc.vector.reciprocal(rz, zs)
            # e transposed back to [s, b] columns for the weighted sum
            ps_e0 = trxp.tile([SC0, BT], BF16, tag="trx")
            nc.tensor.transpose(ps_e0, ebs[:, 0:SC0], idb_s[0:BT, 0:BT])
            ps_e1 = trxp.tile([SC1, BT], BF16, tag="trx")
            nc.tensor.transpose(ps_e1, ebs[:, SC0:S], idb_s[0:BT, 0:BT])
            eT0 = tilep.tile([SC0, BT], BF16, tag="eT0")
            nc.vector.tensor_copy(eT0, ps_e0)
            eT1 = tilep.tile([SC1, BT], BF16, tag="eT1")
            nc.scalar.copy(eT1, ps_e1)
            st.update(eT0=eT0, eT1=eT1, rz=rz)

        def emit_wsum_group(st, g):
            acc, eT0, eT1 = st["acc"], st["eT0"], st["eT1"]
            hn0, hn1 = st["hn0"][g], st["hn1"][g]
            for j in range(NG):
                jb = g * NG + j
                nc.tensor.matmul(acc[:, 2, jb:jb + 1], lhsT=hn0[:, j, :],
                                 rhs=eT0[:, jb:jb + 1], start=False,
                                 stop=False, skip_group_check=True)
                nc.tensor.matmul(acc[:, 2, jb:jb + 1], lhsT=hn1[:, j, :],
                                 rhs=eT1[:, jb:jb + 1], start=False,
                                 stop=True, skip_group_check=True)

        def emit_output(st, tt):
            b0 = tt * BT
            acc, rz = st["acc"], st["rz"]
            oT = tilep.tile([D, BT], F32, tag="oT")
            nc.vector.tensor_copy(oT, acc[:, 2, :])
            ps_ot = trxp.tile([BT, D], F32, tag="trx")
            nc.tensor.transpose(ps_ot, oT, idf_s[0:D, 0:D])
            ofin = tilep.tile([BT, D], F32, tag="ofin")
            nc.vector.tensor_scalar_mul(ofin, ps_ot, rz)
            nc.sync.dma_start(out=out[b0:b0 + BT, :], in_=ofin)

        # ---- software pipeline over tiles: overlap tile t's weighted-sum
        # with tile t+1's load/transpose/MLP/score groups
        st_cur = emit_tile_prep(0)
        for g in range(n_groups):
            emit_group_pass1(st_cur, 0, g)
        emit_softmax(st_cur)
        for tt in range(n_tiles):
            if tt + 1 < n_tiles:
                st_next = emit_tile_prep(tt + 1)
                for g in range(n_groups):
                    emit_wsum_group(st_cur, g)
                    emit_group_pass1(st_next, tt + 1, g)
                emit_output(st_cur, tt)
                emit_softmax(st_next)
                st_cur = st_next
            else:
                for g in range(n_groups):
                    emit_wsum_group(st_cur, g)
                emit_output(st_cur, tt)

    nc.compile()
    return nc


_CACHE = {}


def _get_nc(Bc=256, BT=128, NG=8):
    key = (Bc, BT, NG)
    if key not in _CACHE:
        _CACHE[key] = build_nc(Bc, BT, NG)
    return _CACHE[key]


def make_in_maps(target_item, history_sequence, mask, W1, b1, W2, b2,
                 n_cores=N_CORES):
    """Host-side prep: factored weights, penalty array, per-core shards."""
    f32 = np.float32
    bf16 = ml_dtypes.bfloat16
    W1 = np.asarray(W1, f32)
    wt = (W1[0:D] + W1[2 * D:3 * D]).astype(bf16)
    wh = (W1[D:2 * D] - W1[2 * D:3 * D]).astype(bf16)
    wp = W1[3 * D:4 * D].astype(bf16)
    b1v = np.asarray(b1, f32).reshape(H)
    b1c = np.concatenate([b1v, b1v]).reshape(2 * H, 1)
    w2v = np.asarray(W2, f32).reshape(H)
    w2c = np.concatenate([w2v, w2v]).astype(bf16).reshape(2 * H, 1)
    idf = np.eye(128, dtype=f32)
    idb = np.eye(128).astype(bf16)
    pen_full = ((np.asarray(mask, f32) - 1.0) * 1e9).astype(f32)
    tgt_full = np.asarray(target_item, f32)
    hist_full = np.asarray(history_sequence, f32).astype(bf16)

    shared = dict(wt=wt, wh=wh, wp=wp, b1=b1c, w2=w2c, idf=idf, idb=idb)
    Bc = tgt_full.shape[0] // n_cores
    in_maps = []
    for c in range(n_cores):
        sl = slice(c * Bc, (c + 1) * Bc)
        in_maps.append(dict(hist=hist_full[sl], tgt=tgt_full[sl],
                            pen=pen_full[sl], **shared))
    return in_maps


def kernel(target_item, history_sequence, mask, W1, b1, W2, b2):
    nc = _get_nc()
    in_maps = make_in_maps(target_item, history_sequence, mask, W1, b1, W2, b2)
    res = run_bass_kernel_spmd(nc, in_maps, list(range(N_CORES)))
    return np.concatenate([res.results[c]["out"] for c in range(N_CORES)],
                          axis=0)

